# revision 1
# baseline (speedup 1.0000x reference)
"""Block-causal (block=64) MHA + qkv/out projections on 8 NeuronCores.

Sharding: 8 cores = 2 batches x 4 head-groups (4 heads each).
Per core: qkv projection for its heads, block-causal attention for 4 heads
(processed as 2 head-pairs packed across the 128 partitions), partial output
projection over its 256 channels. Host sums the 4 partials per batch + bias.

On-chip layout is feature-major (transposed): scores are computed transposed
(S^T[k, q] = k . q) so no on-chip transposes are needed anywhere; softmax
denominators (sums over the key/partition axis) come from an all-ones matmul
on the PE, broadcast across 64 partitions. exp runs on ScalarE straight out
of PSUM. The diagonal 128-key tiles are split into two 64-key sub-blocks with
N-restricted matmuls, so block-causality costs no masking ops.
"""

import os

import numpy as np

import concourse.bass as bass
import concourse.tile as tile
from concourse import bacc
from concourse import mybir

B, N, C = 2, 2048, 1024
H, HD = 16, 64
HPC = 4  # heads per core
CSL = HPC * HD  # 256 channel slice per core
QKW = 2 * CSL  # 512: q then k output channels
NCORES = 8
QBLK = 512
NQB = N // QBLK  # 4
NT = N // 128  # 16 seq tiles of 128
SCALE = HD**-0.5
F32 = mybir.dt.float32
F32R = mybir.dt.float32r

USE_F32R = False


def _mm(ap):
    """Matmul operand view: compute in tf32-like float32r for full PE rate."""
    return ap.bitcast(F32R) if USE_F32R else ap


def build_nc():
    nc = bacc.Bacc("TRN2", target_bir_lowering=False, debug=False, num_devices=NCORES)

    xT_d = nc.dram_tensor("xT", [8, 128, N], F32, kind="ExternalInput")
    wqk_d = nc.dram_tensor("wqkT", [8, 128, QKW], F32, kind="ExternalInput")
    wv_d = nc.dram_tensor("wvT", [8, 128, CSL], F32, kind="ExternalInput")
    wp_d = nc.dram_tensor("wpT", [2, 128, C], F32, kind="ExternalInput")
    y_d = nc.dram_tensor("y", [N, C], F32, kind="ExternalOutput")

    with tile.TileContext(nc) as tc:
        with (
            tc.tile_pool(name="persist", bufs=1) as persist,
            tc.tile_pool(name="pt", bufs=2) as pt_pool,
            tc.tile_pool(name="rc", bufs=2) as rc_pool,
            tc.tile_pool(name="yout", bufs=3) as y_pool,
            tc.tile_pool(name="psmm", bufs=2, space="PSUM") as ps_mm,
            tc.tile_pool(name="pssc", bufs=1, space="PSUM") as ps_sc,
            tc.tile_pool(name="psacc", bufs=1, space="PSUM") as ps_acc,
        ):
            # ---- load inputs (one tile per DMA so consumers wait on few sems) ----
            xts = [persist.tile([128, N], F32, tag=f"xt{i}", name=f"xt{i}") for i in range(8)]
            wqks = [persist.tile([128, QKW], F32, tag=f"wqk{i}", name=f"wqk{i}") for i in range(8)]
            wvs = [persist.tile([128, CSL], F32, tag=f"wv{i}", name=f"wv{i}") for i in range(8)]
            wps = [persist.tile([128, C], F32, tag=f"wp{i}", name=f"wp{i}") for i in range(2)]
            for ct in range(8):
                nc.sync.dma_start(out=xts[ct], in_=xT_d[ct])
                nc.sync.dma_start(out=wqks[ct], in_=wqk_d[ct])
                nc.sync.dma_start(out=wvs[ct], in_=wv_d[ct])
            for pr in range(2):
                nc.sync.dma_start(out=wps[pr], in_=wp_d[pr])

            ones_t = persist.tile([128, 128], F32, tag="ones")
            nc.vector.memset(ones_t, 1.0)

            # ---- phase 1: q/k projection, transposed outputs ----
            # qkT tiles: 0 = q heads(0,1), 1 = q heads(2,3), 2 = k(0,1), 3 = k(3,4)
            # within a tile: partitions 0:64 = even head dims, 64:128 = odd head.
            qkT = [persist.tile([128, N], F32, tag=f"qk{t}", name=f"qk{t}") for t in range(4)]
            for dt_ in range(4):
                for nb in range(NQB):
                    ps = ps_mm.tile([128, QBLK], F32, tag="mm")
                    for ct in range(8):
                        nc.tensor.matmul(
                            ps,
                            lhsT=_mm(wqks[ct][:, dt_ * 128 : (dt_ + 1) * 128]),
                            rhs=_mm(xts[ct][:, nb * QBLK : (nb + 1) * QBLK]),
                            start=(ct == 0),
                            stop=(ct == 7),
                        )
                    nc.vector.tensor_copy(
                        out=qkT[dt_][:, nb * QBLK : (nb + 1) * QBLK], in_=ps
                    )

            # ---- phase 2: v projection, natural layout [n, 4*64] ----
            v_sb = [persist.tile([128, CSL], F32, tag=f"v{t}", name=f"v{t}") for t in range(NT)]
            for nt in range(NT):
                ps = ps_mm.tile([128, CSL], F32, tag="mm")
                for ct in range(8):
                    nc.tensor.matmul(
                        ps,
                        lhsT=_mm(xts[ct][:, nt * 128 : (nt + 1) * 128]),
                        rhs=_mm(wvs[ct]),
                        start=(ct == 0),
                        stop=(ct == 7),
                    )
                nc.vector.tensor_copy(out=v_sb[nt], in_=ps)

            # ---- phase 3+4: attention (per 512-query block), then out-proj ----
            PHASES = int(os.environ.get("KERNEL_PHASES", "3"))
            attnT = [persist.tile([128, N], F32, tag=f"at{p}", name=f"at{p}") for p in range(2)]
            if PHASES == 1:
                for p in range(2):
                    nc.vector.memset(attnT[p], 0.0)
            for qi in range(NQB if PHASES >= 2 else 0):
                for pair in range(2):
                    qt = qkT[pair]
                    kt_t = qkT[2 + pair]
                    qs = slice(qi * QBLK, (qi + 1) * QBLK)

                    # one PSUM bank per head per accumulator: the psum
                    # accumulation-group tracking cannot mix base-partition-0
                    # and base-partition-64 groups in one bank.
                    at_bA = ps_acc.tile([128, QBLK], F32, tag="atA", name="at_bA")
                    at_bB = ps_acc.tile([128, QBLK], F32, tag="atB", name="at_bB")
                    sm_bA = ps_acc.tile([128, QBLK], F32, tag="smA", name="sm_bA")
                    sm_bB = ps_acc.tile([128, QBLK], F32, tag="smB", name="sm_bB")

                    n_reg = 4 * qi
                    ATT_RECT = os.environ.get("ATT_RECT", "0") == "1"
                    diag_upto = int(os.environ.get("ATT_DIAG_UPTO", "8"))
                    if qi * 2 + pair >= diag_upto:
                        ATT_RECT = True
                    if ATT_RECT:
                        n_reg = 4 * qi + 4  # probe: no diagonal handling at all
                    # per partition-range (head) accumulation-group flags:
                    # the sim/HW psum group model tracks start/stop per
                    # partition range, so each head brackets its own group.
                    n_per_range = n_reg + (0 if ATT_RECT else 4)
                    at_A, at_B, sm_A, sm_B = [0], [0], [0], [0]

                    def fl(cnt, total=n_per_range):
                        i = cnt[0]
                        cnt[0] += 1
                        return dict(start=(i == 0), stop=(i == total - 1))

                    # fully-causal key tiles: whole [128k x 512q] rectangles
                    for kt in range(n_reg):
                        ks = slice(kt * 128, (kt + 1) * 128)
                        psA = ps_sc.tile([128, QBLK], F32, tag="sA")
                        psB = ps_sc.tile([128, QBLK], F32, tag="sB")
                        nc.tensor.matmul(
                            psA, lhsT=_mm(kt_t[0:64, ks]), rhs=_mm(qt[0:64, qs]),
                            start=True, stop=True,
                        )
                        nc.tensor.matmul(
                            psB, lhsT=_mm(kt_t[64:128, ks]), rhs=_mm(qt[64:128, qs]),
                            start=True, stop=True,
                        )
                        pA = pt_pool.tile([128, QBLK], F32, tag="pA")
                        pB = pt_pool.tile([128, QBLK], F32, tag="pB")
                        nc.scalar.activation(
                            out=pA, in_=psA, func=mybir.ActivationFunctionType.Exp,
                            scale=SCALE,
                        )
                        nc.scalar.activation(
                            out=pB, in_=psB, func=mybir.ActivationFunctionType.Exp,
                            scale=SCALE,
                        )
                        vA = v_sb[kt][:, pair * 128 : pair * 128 + 64]
                        vB = v_sb[kt][:, pair * 128 + 64 : pair * 128 + 128]
                        nc.tensor.matmul(
                            at_bA[0:64, :], lhsT=_mm(vA), rhs=_mm(pA), **fl(at_A)
                        )
                        nc.tensor.matmul(
                            at_bB[64:128, :], lhsT=_mm(vB), rhs=_mm(pB), **fl(at_B)
                        )
                        nc.tensor.matmul(
                            sm_bA[0:64, :], lhsT=_mm(ones_t[:, 0:64]), rhs=_mm(pA),
                            **fl(sm_A),
                        )
                        nc.tensor.matmul(
                            sm_bB[64:128, :], lhsT=_mm(ones_t[:, 64:128]), rhs=_mm(pB),
                            **fl(sm_B),
                        )

                    # diagonal key tiles: two 64-key sub-blocks, N-restricted
                    for j in ([] if ATT_RECT else range(4)):
                        kt = 4 * qi + j
                        q0 = 128 * j  # first allowed q offset for keys [0,64)
                        q1 = 128 * j + 64  # for keys [64,128)
                        if os.environ.get("ATT_DIAG_FULLN", "0") == "1":
                            q0 = q1 = 0  # probe: quadrant MMs, full N
                        k0 = slice(kt * 128, kt * 128 + 64)
                        k1 = slice(kt * 128 + 64, (kt + 1) * 128)
                        psA = ps_sc.tile([128, QBLK], F32, tag="sA")
                        psB = ps_sc.tile([128, QBLK], F32, tag="sB")
                        qsl0 = slice(qi * QBLK + q0, (qi + 1) * QBLK)
                        qsl1 = slice(qi * QBLK + q1, (qi + 1) * QBLK)
                        pA = pt_pool.tile([128, QBLK], F32, tag="pA")
                        pB = pt_pool.tile([128, QBLK], F32, tag="pB")
                        for ph, ps_s, p_s in ((0, psA, pA), (64, psB, pB)):
                            hd_sl = slice(ph, ph + 64)
                            # sub1 computes from q0 (not q1) so the bank is
                            # fully written and ONE exp covers both halves —
                            # two exps would read the bank while the second
                            # sub-MM still writes it (fatal PSUM collision).
                            nc.tensor.matmul(
                                ps_s[0:64, q0:QBLK], lhsT=_mm(kt_t[hd_sl, k0]),
                                rhs=_mm(qt[hd_sl, qsl0]), start=True, stop=True,
                            )
                            nc.tensor.matmul(
                                ps_s[64:128, q0:QBLK], lhsT=_mm(kt_t[hd_sl, k1]),
                                rhs=_mm(qt[hd_sl, qsl0]), start=True, stop=True,
                            )
                            nc.scalar.activation(
                                out=p_s[:, q0:QBLK], in_=ps_s[:, q0:QBLK],
                                func=mybir.ActivationFunctionType.Exp, scale=SCALE,
                            )
                            # zero the disallowed corner (keys k1 x queries
                            # [q0,q1)) so PV/sum can run as single K=128
                            # matmuls. Two row-split accumulating MMs would
                            # drain concurrently into the same PSUM cells —
                            # a fatal collision on hardware.
                            nc.gpsimd.memset(p_s[64:128, q0:q1], 0.0)
                        for ph, p_s, at_c, sm_c, at_b, sm_b in (
                            (0, pA, at_A, sm_A, at_bA, sm_bA),
                            (64, pB, at_B, sm_B, at_bB, sm_bB),
                        ):
                            vc = pair * 128 + ph  # head col offset: 0/64
                            nc.tensor.matmul(
                                at_b[ph : ph + 64, q0:QBLK],
                                lhsT=_mm(v_sb[kt][:, vc : vc + 64]),
                                rhs=_mm(p_s[:, q0:QBLK]), **fl(at_c),
                            )
                            nc.tensor.matmul(
                                sm_b[ph : ph + 64, q0:QBLK],
                                lhsT=_mm(ones_t[:, ph : ph + 64]),
                                rhs=_mm(p_s[:, q0:QBLK]), **fl(sm_c),
                            )

                    # normalize: attnT[:, qblock] = at * (1 / sm), per head half
                    recip = rc_pool.tile([128, QBLK], F32, tag="rc")
                    nc.vector.reciprocal(out=recip[0:64, :], in_=sm_bA[0:64, :])
                    nc.vector.reciprocal(out=recip[64:128, :], in_=sm_bB[64:128, :])
                    nc.vector.tensor_mul(
                        out=attnT[pair][0:64, qs], in0=at_bA[0:64, :], in1=recip[0:64, :]
                    )
                    nc.vector.tensor_mul(
                        out=attnT[pair][64:128, qs], in0=at_bB[64:128, :],
                        in1=recip[64:128, :],
                    )

                # output projection for this query block's 4 row tiles
                for nt in (range(4 * qi, 4 * qi + 4) if PHASES >= 3 else []):
                    ysb = y_pool.tile([128, C], F32, tag="y")
                    for cb in range(2):
                        psy = ps_mm.tile([128, QBLK], F32, tag="mm")
                        for pr in range(2):
                            nc.tensor.matmul(
                                psy,
                                lhsT=_mm(attnT[pr][:, nt * 128 : (nt + 1) * 128]),
                                rhs=_mm(wps[pr][:, cb * QBLK : (cb + 1) * QBLK]),
                                start=(pr == 0),
                                stop=(pr == 1),
                            )
                        nc.vector.tensor_copy(
                            out=ysb[:, cb * QBLK : (cb + 1) * QBLK], in_=psy
                        )
                    nc.sync.dma_start(out=y_d[nt * 128 : (nt + 1) * 128, :], in_=ysb)

            if PHASES < 3:
                for nt in range(NT):
                    ysb = y_pool.tile([128, C], F32, tag="y", name="ysb_fb")
                    for cb in range(2):
                        psy = ps_mm.tile([128, QBLK], F32, tag="mm", name="psy_fb")
                        for pr in range(2):
                            nc.tensor.matmul(
                                psy,
                                lhsT=_mm(attnT[pr][:, nt * 128 : (nt + 1) * 128]),
                                rhs=_mm(wps[pr][:, cb * QBLK : (cb + 1) * QBLK]),
                                start=(pr == 0),
                                stop=(pr == 1),
                            )
                        nc.vector.tensor_copy(
                            out=ysb[:, cb * QBLK : (cb + 1) * QBLK], in_=psy
                        )
                    nc.sync.dma_start(out=y_d[nt * 128 : (nt + 1) * 128, :], in_=ysb)

    return nc


def _shard_inputs(x, w_qkv, w_proj):
    x = np.ascontiguousarray(np.asarray(x, dtype=np.float32))
    w_qkv = np.asarray(w_qkv, dtype=np.float32)
    w_proj = np.asarray(w_proj, dtype=np.float32)
    xT = [np.ascontiguousarray(x[b].T).reshape(8, 128, N) for b in range(B)]
    in_maps = []
    for c in range(NCORES):
        b, g = divmod(c, 4)
        r0 = 64 * HPC * g  # 256 * g
        wq = w_qkv[r0 : r0 + CSL, :]
        wk = w_qkv[C + r0 : C + r0 + CSL, :]
        wvs = w_qkv[2 * C + r0 : 2 * C + r0 + CSL, :]
        wqkT = np.ascontiguousarray(np.concatenate([wq, wk], axis=0).T)
        wvT = np.ascontiguousarray(wvs.T)
        wpT = np.ascontiguousarray(w_proj[:, r0 : r0 + CSL].T)
        in_maps.append(
            {
                "xT": xT[b],
                "wqkT": wqkT.reshape(8, 128, QKW),
                "wvT": wvT.reshape(8, 128, CSL),
                "wpT": wpT.reshape(2, 128, C),
            }
        )
    return in_maps


def run(x, w_qkv, w_proj, b_proj, trace=False, **spmd_kwargs):
    from concourse.bass_utils import run_bass_kernel_spmd

    in_maps = _shard_inputs(x, w_qkv, w_proj)
    nc = build_nc()
    nc.finalize()
    res = run_bass_kernel_spmd(
        nc, in_maps, core_ids=list(range(NCORES)), trace=trace, **spmd_kwargs
    )
    y = np.zeros((B, N, C), np.float32)
    for c in range(NCORES):
        y[c // 4] += res.results[c]["y"]
    y += np.asarray(b_proj, dtype=np.float32)[None, None, :]
    return y, res


def kernel(x, w_qkv, w_proj, b_proj):
    y, _ = run(x, w_qkv, w_proj, b_proj, trace=False)
    return y



# revision 4
# speedup vs baseline: 2.2165x; 2.2165x over previous
"""Block-causal (block=64) MHA + qkv/out projections on 8 NeuronCores.

Sharding: 8 cores = 2 batches x 4 head-groups (4 heads each).
Per core: qkv projection for its heads, block-causal attention for 4 heads
(processed as 2 head-pairs packed across the 128 partitions), partial output
projection over its 256 channels. Host sums the 4 partials per batch + bias.

On-chip layout is feature-major (transposed): scores are computed transposed
(S^T[k, q] = k . q) so no on-chip transposes are needed anywhere; softmax
denominators (sums over the key/partition axis) come from an all-ones matmul
on the PE, broadcast across 64 partitions. exp runs on ScalarE straight out
of PSUM. The diagonal 128-key tiles are split into two 64-key sub-blocks with
N-restricted matmuls, so block-causality costs no masking ops.

All matmul operands are bf16 (PSUM accumulation stays fp32): full PE rate
(1 cycle/row vs 4 for fp32) and half the DMA/SBUF traffic. Inputs are
converted host-side; output partials stay fp32.
"""

import ml_dtypes
import numpy as np

import concourse.bass as bass
import concourse.tile as tile
from concourse import bacc
from concourse import mybir

B, N, C = 2, 2048, 1024
H, HD = 16, 64
HPC = 4  # heads per core
CSL = HPC * HD  # 256 channel slice per core
QKW = 2 * CSL  # 512: q then k output channels
NCORES = 8
QBLK = 512
NQB = N // QBLK  # 4
NT = N // 128  # 16 seq tiles of 128
SCALE = HD**-0.5
F32 = mybir.dt.float32
BF16 = mybir.dt.bfloat16
NP_BF16 = ml_dtypes.bfloat16


def build_nc():
    nc = bacc.Bacc("TRN2", target_bir_lowering=False, debug=False, num_devices=NCORES)

    xT_d = nc.dram_tensor("xT", [8, 128, N], BF16, kind="ExternalInput")
    wqk_d = nc.dram_tensor("wqkT", [8, 128, QKW], BF16, kind="ExternalInput")
    wv_d = nc.dram_tensor("wvT", [8, 128, CSL], BF16, kind="ExternalInput")
    wp_d = nc.dram_tensor("wpT", [2, 128, C], BF16, kind="ExternalInput")
    y_d = nc.dram_tensor("y", [N, C], F32, kind="ExternalOutput")

    with tile.TileContext(nc) as tc:
        with (
            tc.tile_pool(name="persist", bufs=1) as persist,
            tc.tile_pool(name="pt", bufs=2) as pt_pool,
            tc.tile_pool(name="rc", bufs=2) as rc_pool,
            tc.tile_pool(name="yout", bufs=3) as y_pool,
            tc.tile_pool(name="psmm", bufs=2, space="PSUM") as ps_mm,
            tc.tile_pool(name="pssc", bufs=1, space="PSUM") as ps_sc,
            tc.tile_pool(name="psacc", bufs=1, space="PSUM") as ps_acc,
        ):
            # ---- load inputs (one tile per DMA so consumers wait on few sems) ----
            xts = [persist.tile([128, N], BF16, tag=f"xt{i}", name=f"xt{i}") for i in range(8)]
            wqks = [persist.tile([128, QKW], BF16, tag=f"wqk{i}", name=f"wqk{i}") for i in range(8)]
            wvs = [persist.tile([128, CSL], BF16, tag=f"wv{i}", name=f"wv{i}") for i in range(8)]
            wps = [persist.tile([128, C], BF16, tag=f"wp{i}", name=f"wp{i}") for i in range(2)]
            for ct in range(8):
                nc.sync.dma_start(out=xts[ct], in_=xT_d[ct])
                nc.sync.dma_start(out=wqks[ct], in_=wqk_d[ct])
                nc.sync.dma_start(out=wvs[ct], in_=wv_d[ct])
            for pr in range(2):
                nc.sync.dma_start(out=wps[pr], in_=wp_d[pr])

            ones_t = persist.tile([128, 128], BF16, tag="ones")
            nc.vector.memset(ones_t, 1.0)

            # ---- phase 1: q/k projection, transposed outputs ----
            # qkT tiles: 0 = q heads(0,1), 1 = q heads(2,3), 2 = k(0,1), 3 = k(3,4)
            # within a tile: partitions 0:64 = even head dims, 64:128 = odd head.
            qkT = [persist.tile([128, N], BF16, tag=f"qk{t}", name=f"qk{t}") for t in range(4)]
            for dt_ in range(4):
                for nb in range(NQB):
                    ps = ps_mm.tile([128, QBLK], F32, tag="mm")
                    for ct in range(8):
                        nc.tensor.matmul(
                            ps,
                            lhsT=wqks[ct][:, dt_ * 128 : (dt_ + 1) * 128],
                            rhs=xts[ct][:, nb * QBLK : (nb + 1) * QBLK],
                            start=(ct == 0),
                            stop=(ct == 7),
                        )
                    nc.vector.tensor_copy(
                        out=qkT[dt_][:, nb * QBLK : (nb + 1) * QBLK], in_=ps
                    )

            # ---- phase 2: v projection, natural layout [n, 4*64] ----
            v_sb = [persist.tile([128, CSL], BF16, tag=f"v{t}", name=f"v{t}") for t in range(NT)]
            for nt in range(NT):
                ps = ps_mm.tile([128, CSL], F32, tag="mm")
                for ct in range(8):
                    nc.tensor.matmul(
                        ps,
                        lhsT=xts[ct][:, nt * 128 : (nt + 1) * 128],
                        rhs=wvs[ct],
                        start=(ct == 0),
                        stop=(ct == 7),
                    )
                nc.vector.tensor_copy(out=v_sb[nt], in_=ps)

            # ---- phase 3+4: attention (per 512-query block), then out-proj ----
            attnT = [persist.tile([128, N], BF16, tag=f"at{p}", name=f"at{p}") for p in range(2)]
            for qi in range(NQB):
                for pair in range(2):
                    qt = qkT[pair]
                    kt_t = qkT[2 + pair]
                    qs = slice(qi * QBLK, (qi + 1) * QBLK)

                    # one PSUM bank per head per accumulator: the psum
                    # accumulation-group tracking cannot mix base-partition-0
                    # and base-partition-64 groups in one bank.
                    at_bA = ps_acc.tile([128, QBLK], F32, tag="atA", name="at_bA")
                    at_bB = ps_acc.tile([128, QBLK], F32, tag="atB", name="at_bB")
                    sm_bA = ps_acc.tile([128, QBLK], F32, tag="smA", name="sm_bA")
                    sm_bB = ps_acc.tile([128, QBLK], F32, tag="smB", name="sm_bB")

                    n_reg = 4 * qi
                    # per partition-range (head) accumulation-group flags:
                    # the sim/HW psum group model tracks start/stop per
                    # partition range, so each head brackets its own group.
                    n_per_range = n_reg + 4
                    at_A, at_B, sm_A, sm_B = [0], [0], [0], [0]

                    def fl(cnt, total=n_per_range):
                        i = cnt[0]
                        cnt[0] += 1
                        return dict(start=(i == 0), stop=(i == total - 1))

                    # fully-causal key tiles: whole [128k x 512q] rectangles
                    for kt in range(n_reg):
                        ks = slice(kt * 128, (kt + 1) * 128)
                        psA = ps_sc.tile([128, QBLK], F32, tag="sA")
                        psB = ps_sc.tile([128, QBLK], F32, tag="sB")
                        nc.tensor.matmul(
                            psA, lhsT=kt_t[0:64, ks], rhs=qt[0:64, qs],
                            start=True, stop=True,
                        )
                        nc.tensor.matmul(
                            psB, lhsT=kt_t[64:128, ks], rhs=qt[64:128, qs],
                            start=True, stop=True,
                        )
                        pA = pt_pool.tile([128, QBLK], BF16, tag="pA")
                        pB = pt_pool.tile([128, QBLK], BF16, tag="pB")
                        nc.scalar.activation(
                            out=pA, in_=psA, func=mybir.ActivationFunctionType.Exp,
                            scale=SCALE,
                        )
                        nc.scalar.activation(
                            out=pB, in_=psB, func=mybir.ActivationFunctionType.Exp,
                            scale=SCALE,
                        )
                        vA = v_sb[kt][:, pair * 128 : pair * 128 + 64]
                        vB = v_sb[kt][:, pair * 128 + 64 : pair * 128 + 128]
                        nc.tensor.matmul(
                            at_bA[0:64, :], lhsT=vA, rhs=pA, **fl(at_A)
                        )
                        nc.tensor.matmul(
                            at_bB[64:128, :], lhsT=vB, rhs=pB, **fl(at_B)
                        )
                        nc.tensor.matmul(
                            sm_bA[0:64, :], lhsT=ones_t[:, 0:64], rhs=pA,
                            **fl(sm_A),
                        )
                        nc.tensor.matmul(
                            sm_bB[64:128, :], lhsT=ones_t[:, 64:128], rhs=pB,
                            **fl(sm_B),
                        )

                    # diagonal key tiles: two 64-key sub-blocks, N-restricted
                    for j in range(4):
                        kt = 4 * qi + j
                        q0 = 128 * j  # first allowed q offset for keys [0,64)
                        q1 = 128 * j + 64  # for keys [64,128)
                        k0 = slice(kt * 128, kt * 128 + 64)
                        k1 = slice(kt * 128 + 64, (kt + 1) * 128)
                        psA = ps_sc.tile([128, QBLK], F32, tag="sA")
                        psB = ps_sc.tile([128, QBLK], F32, tag="sB")
                        qsl0 = slice(qi * QBLK + q0, (qi + 1) * QBLK)
                        pA = pt_pool.tile([128, QBLK], BF16, tag="pA")
                        pB = pt_pool.tile([128, QBLK], BF16, tag="pB")
                        for ph, ps_s, p_s in ((0, psA, pA), (64, psB, pB)):
                            hd_sl = slice(ph, ph + 64)
                            # sub1 computes from q0 (not q1) so the bank is
                            # fully written and ONE exp covers both halves —
                            # two exps would read the bank while the second
                            # sub-MM still writes it (fatal PSUM collision).
                            nc.tensor.matmul(
                                ps_s[0:64, q0:QBLK], lhsT=kt_t[hd_sl, k0],
                                rhs=qt[hd_sl, qsl0], start=True, stop=True,
                            )
                            nc.tensor.matmul(
                                ps_s[64:128, q0:QBLK], lhsT=kt_t[hd_sl, k1],
                                rhs=qt[hd_sl, qsl0], start=True, stop=True,
                            )
                            nc.scalar.activation(
                                out=p_s[:, q0:QBLK], in_=ps_s[:, q0:QBLK],
                                func=mybir.ActivationFunctionType.Exp, scale=SCALE,
                            )
                            # zero the disallowed corner (keys k1 x queries
                            # [q0,q1)) so PV/sum can run as single K=128
                            # matmuls. Two row-split accumulating MMs would
                            # drain concurrently into the same PSUM cells —
                            # a fatal collision on hardware.
                            nc.gpsimd.memset(p_s[64:128, q0:q1], 0.0)
                        for ph, p_s, at_c, sm_c, at_b, sm_b in (
                            (0, pA, at_A, sm_A, at_bA, sm_bA),
                            (64, pB, at_B, sm_B, at_bB, sm_bB),
                        ):
                            vc = pair * 128 + ph  # head col offset: 0/64
                            nc.tensor.matmul(
                                at_b[ph : ph + 64, q0:QBLK],
                                lhsT=v_sb[kt][:, vc : vc + 64],
                                rhs=p_s[:, q0:QBLK], **fl(at_c),
                            )
                            nc.tensor.matmul(
                                sm_b[ph : ph + 64, q0:QBLK],
                                lhsT=ones_t[:, ph : ph + 64],
                                rhs=p_s[:, q0:QBLK], **fl(sm_c),
                            )

                    # normalize: attnT[:, qblock] = at * (1 / sm), per head half.
                    # reciprocal_approx_fast (custom DVE) only works from
                    # base_partition 0 — gather B's sums into the unused rows
                    # 64:128 of A's bank and run one full-tile reciprocal.
                    recip = rc_pool.tile([128, QBLK], F32, tag="rc")
                    nc.vector.tensor_copy(
                        out=sm_bA[64:128, :], in_=sm_bB[64:128, :]
                    )
                    nc.vector.reciprocal_approx_fast(out=recip, in_=sm_bA)
                    nc.vector.tensor_mul(
                        out=attnT[pair][0:64, qs], in0=at_bA[0:64, :], in1=recip[0:64, :]
                    )
                    nc.vector.tensor_mul(
                        out=attnT[pair][64:128, qs], in0=at_bB[64:128, :],
                        in1=recip[64:128, :],
                    )

                # output projection for this query block's 4 row tiles
                for nt in range(4 * qi, 4 * qi + 4):
                    ysb = y_pool.tile([128, C], F32, tag="y")
                    for cb in range(2):
                        psy = ps_mm.tile([128, QBLK], F32, tag="mm")
                        for pr in range(2):
                            nc.tensor.matmul(
                                psy,
                                lhsT=attnT[pr][:, nt * 128 : (nt + 1) * 128],
                                rhs=wps[pr][:, cb * QBLK : (cb + 1) * QBLK],
                                start=(pr == 0),
                                stop=(pr == 1),
                            )
                        nc.vector.tensor_copy(
                            out=ysb[:, cb * QBLK : (cb + 1) * QBLK], in_=psy
                        )
                    nc.sync.dma_start(out=y_d[nt * 128 : (nt + 1) * 128, :], in_=ysb)

    return nc


def _shard_inputs(x, w_qkv, w_proj):
    x = np.ascontiguousarray(np.asarray(x, dtype=np.float32))
    w_qkv = np.asarray(w_qkv, dtype=np.float32)
    w_proj = np.asarray(w_proj, dtype=np.float32)
    xT = [
        np.ascontiguousarray(x[b].T).astype(NP_BF16).reshape(8, 128, N)
        for b in range(B)
    ]
    in_maps = []
    for c in range(NCORES):
        b, g = divmod(c, 4)
        r0 = 64 * HPC * g  # 256 * g
        wq = w_qkv[r0 : r0 + CSL, :]
        wk = w_qkv[C + r0 : C + r0 + CSL, :]
        wvs = w_qkv[2 * C + r0 : 2 * C + r0 + CSL, :]
        wqkT = np.ascontiguousarray(np.concatenate([wq, wk], axis=0).T).astype(NP_BF16)
        wvT = np.ascontiguousarray(wvs.T).astype(NP_BF16)
        wpT = np.ascontiguousarray(w_proj[:, r0 : r0 + CSL].T).astype(NP_BF16)
        in_maps.append(
            {
                "xT": xT[b],
                "wqkT": wqkT.reshape(8, 128, QKW),
                "wvT": wvT.reshape(8, 128, CSL),
                "wpT": wpT.reshape(2, 128, C),
            }
        )
    return in_maps


def run(x, w_qkv, w_proj, b_proj, trace=False, **spmd_kwargs):
    from concourse.bass_utils import run_bass_kernel_spmd

    in_maps = _shard_inputs(x, w_qkv, w_proj)
    nc = build_nc()
    nc.finalize()
    res = run_bass_kernel_spmd(
        nc, in_maps, core_ids=list(range(NCORES)), trace=trace, **spmd_kwargs
    )
    y = np.zeros((B, N, C), np.float32)
    for c in range(NCORES):
        y[c // 4] += res.results[c]["y"]
    y += np.asarray(b_proj, dtype=np.float32)[None, None, :]
    return y, res


def kernel(x, w_qkv, w_proj, b_proj):
    y, _ = run(x, w_qkv, w_proj, b_proj, trace=False)
    return y


# revision 5
# speedup vs baseline: 2.9735x; 1.3415x over previous
"""Block-causal (block=64) MHA + qkv/out projections on 8 NeuronCores.

Sharding: 8 cores = 2 batches x 4 head-groups (4 heads each).
Per core: qkv projection for its heads, block-causal attention for 4 heads
(processed as 2 head-pairs packed across the 128 partitions), partial output
projection over its 256 channels. Host sums the 4 partials per batch + bias.

On-chip layout is feature-major (transposed): scores are computed transposed
(S^T[k, q] = k . q) so no on-chip transposes are needed anywhere. All matmul
operands are bf16 (PSUM accumulation stays fp32): full PE rate and half the
DMA traffic.

Schedule: the attention stream is software-pipelined one key-tile ahead
(scores of tile i+1 issue before the PV of tile i) so ScalarE's exp — the
attention pacer — never starves. Softmax denominators are accumulated OFF the
PE (DVE for head A, GpSimd for head B) and reduced by a single ones-matmul
per head per query block. Projection and out-projection matmuls are emitted
as filler between attention key tiles so the PE stays busy while ScalarE
exponentiates; DMA loads are chunked across queues.
"""

import ml_dtypes
import numpy as np

import concourse.bass as bass
import concourse.tile as tile
from concourse import bacc
from concourse import mybir

B, N, C = 2, 2048, 1024
H, HD = 16, 64
HPC = 4  # heads per core
CSL = HPC * HD  # 256 channel slice per core
QKW = 2 * CSL  # 512: q then k output channels
NCORES = 8
QBLK = 512
NQB = N // QBLK  # 4
NT = N // 128  # 16 seq tiles of 128
SCALE = HD**-0.5
F32 = mybir.dt.float32
BF16 = mybir.dt.bfloat16
NP_BF16 = ml_dtypes.bfloat16
EXP = mybir.ActivationFunctionType.Exp


def build_nc():
    nc = bacc.Bacc("TRN2", target_bir_lowering=False, debug=False, num_devices=NCORES)

    xT_d = nc.dram_tensor("xT", [8, 128, N], BF16, kind="ExternalInput")
    wqk_d = nc.dram_tensor("wqkT", [8, 128, QKW], BF16, kind="ExternalInput")
    wv_d = nc.dram_tensor("wvT", [8, 128, CSL], BF16, kind="ExternalInput")
    wp_d = nc.dram_tensor("wpT", [2, 128, C], BF16, kind="ExternalInput")
    y_d = nc.dram_tensor("y", [N, C], F32, kind="ExternalOutput")

    with tile.TileContext(nc) as tc:
        with (
            tc.tile_pool(name="persist", bufs=1) as persist,
            tc.tile_pool(name="pt", bufs=3) as pt_pool,
            tc.tile_pool(name="pacc", bufs=2) as pacc_pool,
            tc.tile_pool(name="pb", bufs=2) as pb_pool,
            tc.tile_pool(name="rc", bufs=2) as rc_pool,
            tc.tile_pool(name="yout", bufs=4) as y_pool,
            tc.tile_pool(name="psmm", bufs=2, space="PSUM") as ps_mm,
            tc.tile_pool(name="pssc", bufs=2, space="PSUM") as ps_sc,
            tc.tile_pool(name="psacc", bufs=1, space="PSUM") as ps_acc,
        ):
            # ---- load inputs, chunked so several DMA queues run in parallel
            xts = [persist.tile([128, N], BF16, tag=f"xt{i}", name=f"xt{i}") for i in range(8)]
            wqks = [persist.tile([128, QKW], BF16, tag=f"wqk{i}", name=f"wqk{i}") for i in range(8)]
            wvs = [persist.tile([128, CSL], BF16, tag=f"wv{i}", name=f"wv{i}") for i in range(8)]
            wps = [persist.tile([128, C], BF16, tag=f"wp{i}", name=f"wp{i}") for i in range(2)]
            # first the slices phase-0 needs (x cols 0:512 + qk weights + v weights)
            for ct in range(8):
                nc.sync.dma_start(out=xts[ct][:, 0:QBLK], in_=xT_d[ct][:, 0:QBLK])
                nc.sync.dma_start(out=wqks[ct], in_=wqk_d[ct])
            for ct in range(8):
                nc.sync.dma_start(out=wvs[ct], in_=wv_d[ct])
            for nb in range(1, NQB):
                for ct in range(8):
                    nc.sync.dma_start(
                        out=xts[ct][:, nb * QBLK : (nb + 1) * QBLK],
                        in_=xT_d[ct][:, nb * QBLK : (nb + 1) * QBLK],
                    )
            for pr in range(2):
                nc.sync.dma_start(out=wps[pr], in_=wp_d[pr])

            ones_t = persist.tile([128, 64], BF16, tag="ones")
            nc.vector.memset(ones_t, 1.0)

            qkT = [persist.tile([128, N], BF16, tag=f"qk{t}", name=f"qk{t}") for t in range(4)]
            v_sb = [persist.tile([128, CSL], BF16, tag=f"v{t}", name=f"v{t}") for t in range(NT)]
            attnT = [persist.tile([128, N], BF16, tag=f"at{p}", name=f"at{p}") for p in range(2)]

            # ---- work units: projections / out-projections (PE filler) ----
            def qk_unit(dt_, nb):
                def emit():
                    ps = ps_mm.tile([128, QBLK], F32, tag="mm")
                    for ct in range(8):
                        nc.tensor.matmul(
                            ps,
                            lhsT=wqks[ct][:, dt_ * 128 : (dt_ + 1) * 128],
                            rhs=xts[ct][:, nb * QBLK : (nb + 1) * QBLK],
                            start=(ct == 0),
                            stop=(ct == 7),
                        )
                    nc.vector.tensor_copy(
                        out=qkT[dt_][:, nb * QBLK : (nb + 1) * QBLK], in_=ps
                    )
                return emit

            def v_unit(nt):
                def emit():
                    ps = ps_mm.tile([128, QBLK], F32, tag="mm")
                    for ct in range(8):
                        nc.tensor.matmul(
                            ps[:, 0:CSL],
                            lhsT=xts[ct][:, nt * 128 : (nt + 1) * 128],
                            rhs=wvs[ct],
                            start=(ct == 0),
                            stop=(ct == 7),
                        )
                    nc.vector.tensor_copy(out=v_sb[nt], in_=ps[:, 0:CSL])
                return emit

            def op_unit(nt, cb):
                def emit():
                    psy = ps_mm.tile([128, QBLK], F32, tag="mm")
                    for pr in range(2):
                        nc.tensor.matmul(
                            psy,
                            lhsT=attnT[pr][:, nt * 128 : (nt + 1) * 128],
                            rhs=wps[pr][:, cb * QBLK : (cb + 1) * QBLK],
                            start=(pr == 0),
                            stop=(pr == 1),
                        )
                    ysb = y_pool.tile([128, QBLK], F32, tag="y")
                    nc.vector.tensor_copy(out=ysb, in_=psy)
                    nc.sync.dma_start(
                        out=y_d[nt * 128 : (nt + 1) * 128, cb * QBLK : (cb + 1) * QBLK],
                        in_=ysb,
                    )
                return emit

            # filler queue: (needed_before_qi, emit_fn). Units must be emitted
            # before the attention stream of `needed_before_qi` starts.
            filler = []
            for nb in range(1, NQB):
                for nt in range(4 * nb, 4 * nb + 4):
                    filler.append((nb, v_unit(nt)))
                for dt_ in range(4):
                    filler.append((nb, qk_unit(dt_, nb)))

            def drain(n):
                for _ in range(n):
                    if not filler:
                        return
                    filler.pop(0)[1]()

            def drain_required(qi):
                while filler and filler[0][0] <= qi:
                    filler.pop(0)[1]()

            # ---- head: projections needed by attention of qi=0 ----
            for dt_ in range(4):
                qk_unit(dt_, 0)()
            for nt in range(4):
                v_unit(nt)()

            # ---- attention item stream ----
            # item = (qi, pair, kind, kt, j)
            items = []
            for qi in range(NQB):
                for pair in range(2):
                    for kt in range(4 * qi):
                        items.append((qi, pair, "rect", kt, None))
                    for j in range(4):
                        items.append((qi, pair, "diag", 4 * qi + j, j))

            state = {}

            def emit_scores(it):
                qi, pair, kind, kt, j = it
                qt = qkT[pair]
                kt_t = qkT[2 + pair]
                ps = ps_sc.tile([128, 2 * QBLK], F32, tag="s")
                pAB = pt_pool.tile([128, 2 * QBLK], BF16, tag="p")
                if kind == "rect":
                    ks = slice(kt * 128, (kt + 1) * 128)
                    qs = slice(qi * QBLK, (qi + 1) * QBLK)
                    nc.tensor.matmul(
                        ps[:, 0:QBLK], lhsT=kt_t[0:64, ks], rhs=qt[0:64, qs],
                        start=True, stop=True,
                    )
                    nc.tensor.matmul(
                        ps[:, QBLK : 2 * QBLK], lhsT=kt_t[64:128, ks],
                        rhs=qt[64:128, qs], start=True, stop=True,
                    )
                    nc.scalar.activation(out=pAB, in_=ps, func=EXP, scale=SCALE)
                else:
                    q0 = 128 * j
                    k0 = slice(kt * 128, kt * 128 + 64)
                    k1 = slice(kt * 128 + 64, (kt + 1) * 128)
                    qsl0 = slice(qi * QBLK + q0, (qi + 1) * QBLK)
                    for ph, co in ((0, 0), (64, QBLK)):
                        hd_sl = slice(ph, ph + 64)
                        # both sub-MMs span [q0:QBLK] so one exp covers the
                        # fully-written region (see baseline notes on PSUM
                        # write/read collisions).
                        nc.tensor.matmul(
                            ps[0:64, co + q0 : co + QBLK], lhsT=kt_t[hd_sl, k0],
                            rhs=qt[hd_sl, qsl0], start=True, stop=True,
                        )
                        nc.tensor.matmul(
                            ps[64:128, co + q0 : co + QBLK], lhsT=kt_t[hd_sl, k1],
                            rhs=qt[hd_sl, qsl0], start=True, stop=True,
                        )
                        nc.scalar.activation(
                            out=pAB[:, co + q0 : co + QBLK],
                            in_=ps[:, co + q0 : co + QBLK], func=EXP, scale=SCALE,
                        )
                        # zero the disallowed corner (keys [64:128) x queries
                        # [q0:q0+64)) so PV/sum run as single K=128 ops.
                        nc.gpsimd.memset(pAB[64:128, co + q0 : co + q0 + 64], 0.0)
                state[("p", it[:4])] = pAB

            def emit_pv(it, first, flags):
                qi, pair, kind, kt, j = it
                pAB = state.pop(("p", it[:4]))
                q0 = 0 if kind == "rect" else 128 * j
                at_bA, at_bB, paccA, paccB = state[("acc", qi, pair)]
                vA = v_sb[kt][:, pair * 128 : pair * 128 + 64]
                vB = v_sb[kt][:, pair * 128 + 64 : pair * 128 + 128]
                nc.tensor.matmul(
                    at_bA[0:64, q0:QBLK], lhsT=vA, rhs=pAB[:, q0:QBLK], **flags[0]
                )
                nc.tensor.matmul(
                    at_bB[64:128, q0:QBLK], lhsT=vB,
                    rhs=pAB[:, QBLK + q0 : 2 * QBLK], **flags[1]
                )
                # softmax denominators accumulate off-PE: DVE (A) / GpSimd (B)
                if first:
                    nc.vector.tensor_copy(out=paccA, in_=pAB[:, 0:QBLK])
                    nc.gpsimd.tensor_copy(out=paccB, in_=pAB[:, QBLK : 2 * QBLK])
                else:
                    nc.vector.tensor_add(
                        out=paccA[:, q0:QBLK], in0=paccA[:, q0:QBLK],
                        in1=pAB[:, q0:QBLK],
                    )
                    nc.gpsimd.tensor_add(
                        out=paccB[:, q0:QBLK], in0=paccB[:, q0:QBLK],
                        in1=pAB[:, QBLK + q0 : 2 * QBLK],
                    )

            def emit_normalize(qi, pair):
                at_bA, at_bB, paccA, paccB = state.pop(("acc", qi, pair))
                qs = slice(qi * QBLK, (qi + 1) * QBLK)
                pbA = pb_pool.tile([128, QBLK], BF16, tag="pbA")
                pbB = pb_pool.tile([128, QBLK], BF16, tag="pbB")
                nc.vector.tensor_copy(out=pbA, in_=paccA)
                nc.vector.tensor_copy(out=pbB, in_=paccB)
                ps_sm = ps_mm.tile([128, QBLK], F32, tag="mm")
                nc.tensor.matmul(
                    ps_sm[0:64, :], lhsT=ones_t, rhs=pbA, start=True, stop=True
                )
                nc.tensor.matmul(
                    ps_sm[64:128, :], lhsT=ones_t, rhs=pbB, start=True, stop=True
                )
                recip = rc_pool.tile([128, QBLK], F32, tag="rc")
                nc.vector.reciprocal_approx_fast(out=recip, in_=ps_sm)
                nc.vector.tensor_mul(
                    out=attnT[pair][0:64, qs], in0=at_bA[0:64, :], in1=recip[0:64, :]
                )
                nc.vector.tensor_mul(
                    out=attnT[pair][64:128, qs], in0=at_bB[64:128, :],
                    in1=recip[64:128, :],
                )

            # per-(qi,pair) accumulation flag iterators
            def make_flags(qi):
                total = 4 * qi + 4
                cnt = [0, 0]

                def fl():
                    f = []
                    for h in range(2):
                        i = cnt[h]
                        cnt[h] += 1
                        f.append(dict(start=(i % total == 0), stop=(i % total == total - 1)))
                    return f
                return fl

            flag_iters = {}

            def ensure_acc(qi, pair):
                if ("acc", qi, pair) not in state:
                    at_bA = ps_acc.tile([128, QBLK], F32, tag="atA", name="at_bA")
                    at_bB = ps_acc.tile([128, QBLK], F32, tag="atB", name="at_bB")
                    paccA = pacc_pool.tile([128, QBLK], F32, tag="pacA")
                    paccB = pacc_pool.tile([128, QBLK], F32, tag="pacB")
                    state[("acc", qi, pair)] = (at_bA, at_bB, paccA, paccB)
                    flag_iters[(qi, pair)] = make_flags(qi)

            # ---- main pipelined loop: scores run one item ahead of PV ----
            emit_scores(items[0])
            for idx, it in enumerate(items):
                qi, pair, kind, kt, j = it
                ensure_acc(qi, pair)
                nxt = items[idx + 1] if idx + 1 < len(items) else None
                if nxt is not None:
                    if nxt[0] != qi:
                        drain_required(nxt[0])
                    emit_scores(nxt)
                first = (kind == "rect" and kt == 0) or (kind == "diag" and j == 0 and qi == 0)
                emit_pv(it, first, flag_iters[(qi, pair)]())
                drain(1)
                is_last_of_pair = (kind == "diag" and j == 3)
                if is_last_of_pair:
                    emit_normalize(qi, pair)
                    if pair == 1:
                        for nt in range(4 * qi, 4 * qi + 4):
                            for cb in range(2):
                                if qi < NQB - 1:
                                    filler.append((qi + 1, op_unit(nt, cb)))
                                else:
                                    op_unit(nt, cb)()
            drain(len(filler))

    return nc


def _shard_inputs(x, w_qkv, w_proj):
    x = np.ascontiguousarray(np.asarray(x, dtype=np.float32))
    w_qkv = np.asarray(w_qkv, dtype=np.float32)
    w_proj = np.asarray(w_proj, dtype=np.float32)
    xT = [
        np.ascontiguousarray(x[b].T).astype(NP_BF16).reshape(8, 128, N)
        for b in range(B)
    ]
    in_maps = []
    for c in range(NCORES):
        b, g = divmod(c, 4)
        r0 = 64 * HPC * g  # 256 * g
        wq = w_qkv[r0 : r0 + CSL, :]
        wk = w_qkv[C + r0 : C + r0 + CSL, :]
        wvs = w_qkv[2 * C + r0 : 2 * C + r0 + CSL, :]
        wqkT = np.ascontiguousarray(np.concatenate([wq, wk], axis=0).T).astype(NP_BF16)
        wvT = np.ascontiguousarray(wvs.T).astype(NP_BF16)
        wpT = np.ascontiguousarray(w_proj[:, r0 : r0 + CSL].T).astype(NP_BF16)
        in_maps.append(
            {
                "xT": xT[b],
                "wqkT": wqkT.reshape(8, 128, QKW),
                "wvT": wvT.reshape(8, 128, CSL),
                "wpT": wpT.reshape(2, 128, C),
            }
        )
    return in_maps


def run(x, w_qkv, w_proj, b_proj, trace=False, **spmd_kwargs):
    from concourse.bass_utils import run_bass_kernel_spmd

    in_maps = _shard_inputs(x, w_qkv, w_proj)
    nc = build_nc()
    nc.finalize()
    res = run_bass_kernel_spmd(
        nc, in_maps, core_ids=list(range(NCORES)), trace=trace, **spmd_kwargs
    )
    y = np.zeros((B, N, C), np.float32)
    for c in range(NCORES):
        y[c // 4] += res.results[c]["y"]
    y += np.asarray(b_proj, dtype=np.float32)[None, None, :]
    return y, res


def kernel(x, w_qkv, w_proj, b_proj):
    y, _ = run(x, w_qkv, w_proj, b_proj, trace=False)
    return y


# revision 6
# speedup vs baseline: 3.7108x; 1.2480x over previous
"""Block-causal (block=64) MHA + qkv/out projections on 8 NeuronCores.

Sharding: 8 cores = 2 batches x 4 head-groups (4 heads each).
Per core: qkv projection for its heads, block-causal attention for 4 heads
(processed as 2 head-pairs packed across the 128 partitions), partial output
projection over its 256 channels. Host sums the 4 partials per batch + bias.

On-chip layout is feature-major (transposed): scores are computed transposed
(S^T[k, q] = k . q) so no on-chip transposes are needed anywhere. All matmul
operands are bf16 (PSUM accumulation stays fp32): full PE rate and half the
DMA traffic.

Schedule: the attention stream is software-pipelined one key-tile ahead
(scores of tile i+1 issue before the PV of tile i) so ScalarE's exp — the
attention pacer — never starves. Softmax denominators are accumulated OFF the
PE (DVE for head A, GpSimd for head B) and reduced by a single ones-matmul
per head per query block. Projection and out-projection matmuls are emitted
as filler between attention key tiles so the PE stays busy while ScalarE
exponentiates; DMA loads are chunked across queues.
"""

import ml_dtypes
import numpy as np

import concourse.bass as bass
import concourse.tile as tile
from concourse import bacc
from concourse import mybir

B, N, C = 2, 2048, 1024
H, HD = 16, 64
HPC = 4  # heads per core
CSL = HPC * HD  # 256 channel slice per core
QKW = 2 * CSL  # 512: q then k output channels
NCORES = 8
QBLK = 512
NQB = N // QBLK  # 4
NT = N // 128  # 16 seq tiles of 128
SCALE = HD**-0.5
F32 = mybir.dt.float32
BF16 = mybir.dt.bfloat16
NP_BF16 = ml_dtypes.bfloat16
EXP = mybir.ActivationFunctionType.Exp


def build_nc():
    nc = bacc.Bacc("TRN2", target_bir_lowering=False, debug=False, num_devices=NCORES)

    xT_d = nc.dram_tensor("xT", [8, 128, N], BF16, kind="ExternalInput")
    wqk_d = nc.dram_tensor("wqkT", [8, 128, QKW], BF16, kind="ExternalInput")
    wv_d = nc.dram_tensor("wvT", [8, 128, CSL], BF16, kind="ExternalInput")
    wp_d = nc.dram_tensor("wpT", [2, 128, C], BF16, kind="ExternalInput")
    y_d = nc.dram_tensor("y", [N, C], F32, kind="ExternalOutput")

    with tile.TileContext(nc) as tc:
        with (
            tc.tile_pool(name="persist", bufs=1) as persist,
            tc.tile_pool(name="pt", bufs=6) as pt_pool,
            tc.tile_pool(name="rc", bufs=3) as rc_pool,
            tc.tile_pool(name="yout", bufs=4) as y_pool,
            tc.tile_pool(name="psmm", bufs=2, space="PSUM") as ps_mm,
            tc.tile_pool(name="pssc", bufs=2, space="PSUM") as ps_sc,
            tc.tile_pool(name="psacc", bufs=1, space="PSUM") as ps_acc,
        ):
            # ---- load inputs, chunked so several DMA queues run in parallel
            xts = [persist.tile([128, N], BF16, tag=f"xt{i}", name=f"xt{i}") for i in range(8)]
            wqks = [persist.tile([128, QKW], BF16, tag=f"wqk{i}", name=f"wqk{i}") for i in range(8)]
            wvs = [persist.tile([128, CSL], BF16, tag=f"wv{i}", name=f"wv{i}") for i in range(8)]
            wps = [persist.tile([128, C], BF16, tag=f"wp{i}", name=f"wp{i}") for i in range(2)]
            # first the slices phase-0 needs (x cols 0:512 + qk weights + v weights)
            for ct in range(8):
                nc.sync.dma_start(out=xts[ct][:, 0:QBLK], in_=xT_d[ct][:, 0:QBLK])
                nc.sync.dma_start(out=wqks[ct], in_=wqk_d[ct])
            for ct in range(8):
                nc.sync.dma_start(out=wvs[ct], in_=wv_d[ct])
            for nb in range(1, NQB):
                for ct in range(8):
                    nc.sync.dma_start(
                        out=xts[ct][:, nb * QBLK : (nb + 1) * QBLK],
                        in_=xT_d[ct][:, nb * QBLK : (nb + 1) * QBLK],
                    )
            for pr in range(2):
                nc.sync.dma_start(out=wps[pr], in_=wp_d[pr])

            ones_t = persist.tile([128, 64], BF16, tag="ones")
            nc.vector.memset(ones_t, 1.0)

            qkT = [persist.tile([128, N], BF16, tag=f"qk{t}", name=f"qk{t}") for t in range(4)]
            v_sb = [persist.tile([128, CSL], BF16, tag=f"v{t}", name=f"v{t}") for t in range(NT)]
            attnT = [persist.tile([128, N], BF16, tag=f"at{p}", name=f"at{p}") for p in range(2)]

            # ---- work units: projections / out-projections (PE filler) ----
            def qk_unit(dt_, nb):
                def emit():
                    ps = ps_mm.tile([128, QBLK], F32, tag="mm")
                    for ct in range(8):
                        nc.tensor.matmul(
                            ps,
                            lhsT=wqks[ct][:, dt_ * 128 : (dt_ + 1) * 128],
                            rhs=xts[ct][:, nb * QBLK : (nb + 1) * QBLK],
                            start=(ct == 0),
                            stop=(ct == 7),
                        )
                    nc.vector.tensor_copy(
                        out=qkT[dt_][:, nb * QBLK : (nb + 1) * QBLK], in_=ps
                    )
                return emit

            def v_unit(nt):
                def emit():
                    ps = ps_mm.tile([128, QBLK], F32, tag="mm")
                    for ct in range(8):
                        nc.tensor.matmul(
                            ps[:, 0:CSL],
                            lhsT=xts[ct][:, nt * 128 : (nt + 1) * 128],
                            rhs=wvs[ct],
                            start=(ct == 0),
                            stop=(ct == 7),
                        )
                    nc.vector.tensor_copy(out=v_sb[nt], in_=ps[:, 0:CSL])
                return emit

            def op_unit(nt, cb):
                def emit():
                    psy = ps_mm.tile([128, QBLK], F32, tag="mm")
                    for pr in range(2):
                        nc.tensor.matmul(
                            psy,
                            lhsT=attnT[pr][:, nt * 128 : (nt + 1) * 128],
                            rhs=wps[pr][:, cb * QBLK : (cb + 1) * QBLK],
                            start=(pr == 0),
                            stop=(pr == 1),
                        )
                    ysb = y_pool.tile([128, QBLK], F32, tag="y")
                    nc.vector.tensor_copy(out=ysb, in_=psy)
                    nc.sync.dma_start(
                        out=y_d[nt * 128 : (nt + 1) * 128, cb * QBLK : (cb + 1) * QBLK],
                        in_=ysb,
                    )
                return emit

            # filler queue: (needed_before_qi, emit_fn). Units must be emitted
            # before the attention stream of `needed_before_qi` starts.
            filler = []
            for nb in range(1, NQB):
                for nt in range(4 * nb, 4 * nb + 4):
                    filler.append((nb, v_unit(nt)))
                for dt_ in range(4):
                    filler.append((nb, qk_unit(dt_, nb)))

            def drain(n):
                for _ in range(n):
                    if not filler:
                        return
                    filler.pop(0)[1]()

            def drain_required(qi):
                while filler and filler[0][0] <= qi:
                    filler.pop(0)[1]()

            # ---- head: projections needed by attention of qi=0 ----
            for dt_ in range(4):
                qk_unit(dt_, 0)()
            for nt in range(4):
                v_unit(nt)()

            # ---- attention item stream ----
            # item = (qi, pair, kind, kt, j)
            items = []
            for qi in range(NQB):
                for pair in range(2):
                    for kt in range(4 * qi):
                        items.append((qi, pair, "rect", kt, None))
                    for j in range(4):
                        items.append((qi, pair, "diag", 4 * qi + j, j))

            state = {}

            def emit_scores(it):
                qi, pair, kind, kt, j = it
                qt = qkT[pair]
                kt_t = qkT[2 + pair]
                ps = ps_sc.tile([128, 2 * QBLK], F32, tag="s")
                pAB = pt_pool.tile([128, 2 * QBLK], BF16, tag="p")
                if kind == "rect":
                    ks = slice(kt * 128, (kt + 1) * 128)
                    qs = slice(qi * QBLK, (qi + 1) * QBLK)
                    nc.tensor.matmul(
                        ps[:, 0:QBLK], lhsT=kt_t[0:64, ks], rhs=qt[0:64, qs],
                        start=True, stop=True,
                    )
                    nc.tensor.matmul(
                        ps[:, QBLK : 2 * QBLK], lhsT=kt_t[64:128, ks],
                        rhs=qt[64:128, qs], start=True, stop=True,
                    )
                    nc.scalar.activation(out=pAB, in_=ps, func=EXP, scale=SCALE)
                else:
                    q0 = 128 * j
                    k0 = slice(kt * 128, kt * 128 + 64)
                    k1 = slice(kt * 128 + 64, (kt + 1) * 128)
                    qsl0 = slice(qi * QBLK + q0, (qi + 1) * QBLK)
                    for ph, co in ((0, 0), (64, QBLK)):
                        hd_sl = slice(ph, ph + 64)
                        # both sub-MMs span [q0:QBLK] so one exp covers the
                        # fully-written region (see baseline notes on PSUM
                        # write/read collisions).
                        nc.tensor.matmul(
                            ps[0:64, co + q0 : co + QBLK], lhsT=kt_t[hd_sl, k0],
                            rhs=qt[hd_sl, qsl0], start=True, stop=True,
                        )
                        nc.tensor.matmul(
                            ps[64:128, co + q0 : co + QBLK], lhsT=kt_t[hd_sl, k1],
                            rhs=qt[hd_sl, qsl0], start=True, stop=True,
                        )
                    # one exp over both heads' [q0:QBLK] chunks (strided view)
                    ps3 = ps.rearrange("p (c n) -> p c n", c=2)
                    pAB3 = pAB.rearrange("p (c n) -> p c n", c=2)
                    nc.scalar.activation(
                        out=pAB3[:, :, q0:QBLK], in_=ps3[:, :, q0:QBLK],
                        func=EXP, scale=SCALE,
                    )
                    for co in (0, QBLK):
                        # zero the disallowed corner (keys [64:128) x queries
                        # [q0:q0+64)) so PV/sum run as single K=128 ops.
                        nc.gpsimd.memset(pAB[64:128, co + q0 : co + q0 + 64], 0.0)
                state[("p", it[:4])] = pAB

            def emit_pv(it, first, flags):
                qi, pair, kind, kt, j = it
                pAB = state.pop(("p", it[:4]))
                q0 = 0 if kind == "rect" else 128 * j
                at_b, sm_b = state[("acc", qi, pair)]
                vA = v_sb[kt][:, pair * 128 : pair * 128 + 64]
                vB = v_sb[kt][:, pair * 128 + 64 : pair * 128 + 128]
                # heads A/B col-packed into one bank (disjoint partition
                # ranges; the bass group checker can't track mixed-base
                # groups in one bank, but per-element has_written bits can).
                nc.tensor.matmul(
                    at_b[0:64, q0:QBLK], lhsT=vA, rhs=pAB[:, q0:QBLK],
                    skip_group_check=True, **flags[0]
                )
                nc.tensor.matmul(
                    at_b[64:128, q0:QBLK], lhsT=vB,
                    rhs=pAB[:, QBLK + q0 : 2 * QBLK],
                    skip_group_check=True, **flags[1]
                )
                nc.tensor.matmul(
                    sm_b[0:64, q0:QBLK], lhsT=ones_t, rhs=pAB[:, q0:QBLK],
                    skip_group_check=True, **flags[2]
                )
                nc.tensor.matmul(
                    sm_b[64:128, q0:QBLK], lhsT=ones_t,
                    rhs=pAB[:, QBLK + q0 : 2 * QBLK],
                    skip_group_check=True, **flags[3]
                )

            def emit_normalize(qi, pair):
                at_b, sm_b = state.pop(("acc", qi, pair))
                qs = slice(qi * QBLK, (qi + 1) * QBLK)
                recip = rc_pool.tile([128, QBLK], F32, tag="rc")
                nc.vector.reciprocal_approx_fast(out=recip, in_=sm_b)
                nc.vector.tensor_mul(
                    out=attnT[pair][0:64, qs], in0=at_b[0:64, :], in1=recip[0:64, :]
                )
                nc.vector.tensor_mul(
                    out=attnT[pair][64:128, qs], in0=at_b[64:128, :],
                    in1=recip[64:128, :],
                )

            # per-(qi,pair) accumulation flag iterators
            def make_flags(qi):
                total = 4 * qi + 4
                cnt = [0, 0, 0, 0]

                def fl():
                    f = []
                    for h in range(4):
                        i = cnt[h]
                        cnt[h] += 1
                        f.append(dict(start=(i % total == 0), stop=(i % total == total - 1)))
                    return f
                return fl

            flag_iters = {}

            def ensure_acc(qi, pair):
                if ("acc", qi, pair) not in state:
                    at_b = ps_acc.tile([128, QBLK], F32, tag="at", name="at_b")
                    sm_b = ps_acc.tile([128, QBLK], F32, tag="sm", name="sm_b")
                    state[("acc", qi, pair)] = (at_b, sm_b)
                    flag_iters[(qi, pair)] = make_flags(qi)

            # ---- main pipelined loop: scores run one item ahead of PV ----
            emit_scores(items[0])
            for idx, it in enumerate(items):
                qi, pair, kind, kt, j = it
                ensure_acc(qi, pair)
                nxt = items[idx + 1] if idx + 1 < len(items) else None
                if nxt is not None:
                    if nxt[0] != qi:
                        drain_required(nxt[0])
                    emit_scores(nxt)
                first = (kind == "rect" and kt == 0) or (kind == "diag" and j == 0 and qi == 0)
                emit_pv(it, first, flag_iters[(qi, pair)]())
                drain(1)
                is_last_of_pair = (kind == "diag" and j == 3)
                if is_last_of_pair:
                    emit_normalize(qi, pair)
                    if pair == 1:
                        for nt in range(4 * qi, 4 * qi + 4):
                            for cb in range(2):
                                if qi < NQB - 1:
                                    filler.append((qi + 1, op_unit(nt, cb)))
                                else:
                                    op_unit(nt, cb)()
            drain(len(filler))

    return nc


def _shard_inputs(x, w_qkv, w_proj):
    x = np.ascontiguousarray(np.asarray(x, dtype=np.float32))
    w_qkv = np.asarray(w_qkv, dtype=np.float32)
    w_proj = np.asarray(w_proj, dtype=np.float32)
    xT = [
        np.ascontiguousarray(x[b].T).astype(NP_BF16).reshape(8, 128, N)
        for b in range(B)
    ]
    in_maps = []
    for c in range(NCORES):
        b, g = divmod(c, 4)
        r0 = 64 * HPC * g  # 256 * g
        wq = w_qkv[r0 : r0 + CSL, :]
        wk = w_qkv[C + r0 : C + r0 + CSL, :]
        wvs = w_qkv[2 * C + r0 : 2 * C + r0 + CSL, :]
        wqkT = np.ascontiguousarray(np.concatenate([wq, wk], axis=0).T).astype(NP_BF16)
        wvT = np.ascontiguousarray(wvs.T).astype(NP_BF16)
        wpT = np.ascontiguousarray(w_proj[:, r0 : r0 + CSL].T).astype(NP_BF16)
        in_maps.append(
            {
                "xT": xT[b],
                "wqkT": wqkT.reshape(8, 128, QKW),
                "wvT": wvT.reshape(8, 128, CSL),
                "wpT": wpT.reshape(2, 128, C),
            }
        )
    return in_maps


def run(x, w_qkv, w_proj, b_proj, trace=False, **spmd_kwargs):
    from concourse.bass_utils import run_bass_kernel_spmd

    in_maps = _shard_inputs(x, w_qkv, w_proj)
    nc = build_nc()
    nc.finalize()
    res = run_bass_kernel_spmd(
        nc, in_maps, core_ids=list(range(NCORES)), trace=trace, **spmd_kwargs
    )
    y = np.zeros((B, N, C), np.float32)
    for c in range(NCORES):
        y[c // 4] += res.results[c]["y"]
    y += np.asarray(b_proj, dtype=np.float32)[None, None, :]
    return y, res


def kernel(x, w_qkv, w_proj, b_proj):
    y, _ = run(x, w_qkv, w_proj, b_proj, trace=False)
    return y
